# revision 16
# baseline (speedup 1.0000x reference)
"""Cross-attention block (LN -> shared qkv proj -> masked softmax attention
-> out proj) on 8 trn2 NeuronCores.

Sharding: 2-way data parallel over batch x 4-way tensor parallel over heads
(16 heads -> 4 per core). LayerNorm params are folded into the qkv weights
host-side (exact: out = LN_affine(x) @ W + b == LN_plain(x) @ (gamma*W) +
(beta @ W + b)). Each core computes a partial out-projection; ReduceScatters
over each 4-core group sum the partials and leave each core with a shard of
its batch's output, which the host reassembles.

Device layout notes:
 - activations live feature-on-partition ("transposed") for all matmuls;
   LayerNorm runs in natural layout and the result is transposed on the PE.
 - scores are computed transposed [s, q] so the softmax reduction over s can
   ride the P@V matmul: V gets an extra all-ones column producing the
   denominator, and masked keys are handled by zeroing their V rows (and the
   ones column), which is exactly softmax with -inf masked scores. exp() is
   applied without max-subtraction (scores for these inputs are O(5), far
   from overflow; softmax is shift-invariant so the result is identical).
 - matmul operands are bf16 (fp32 PSUM accumulate). fp32/f32r matmuls do not
   register as PE activity for the HAM clock governor, pinning the PE at
   1.2 GHz; bf16 runs warm at 2.4 GHz.
 - attention runs one 512-query block at a time across all heads, so the
   out-projection + ReduceScatter of block 0 overlap block 1's attention.
"""

import numpy as np
import ml_dtypes

import concourse.bass as bass
import concourse.mybir as mybir
import concourse.tile as tile
from concourse import bacc
from concourse.bass_utils import run_bass_kernel_spmd
from concourse.masks import make_identity

B, NQ, S, H, NH = 2, 1024, 4096, 1024, 16
HD = H // NH          # 64
GROUPS = 4            # head-parallel ways per batch
NH_L = NH // GROUPS   # heads per core
DQ = NH_L * HD        # per-core projected dim (256)
EPS = 1e-6
SCALE = 1.0 / float(np.sqrt(HD))

F32 = mybir.dt.float32
BF16 = mybir.dt.bfloat16
U8 = mybir.dt.uint8

KC = H // 128         # feature chunks (8)
MQ = DQ // 128        # per-core projected-dim tiles (2)


def _ln_chunk(nc, pools, x_dram_rows, xnt, jslot):
    """LayerNorm 128 tokens (natural layout) then transpose into
    xnt[:, :, jslot*128:(jslot+1)*128] (feature-on-partition, bf16)."""
    xp, st, ps = pools["x"], pools["st"], pools["mm"]
    x = xp.tile([128, H], F32, tag="x")
    nc.sync.dma_start(out=x, in_=x_dram_rows)
    stats = st.tile([128, 2, 6], F32, tag="st")
    nc.vector.bn_stats(out=stats[:, 0, :], in_=x[:, 0:512])
    nc.vector.bn_stats(out=stats[:, 1, :], in_=x[:, 512:1024])
    mv = st.tile([128, 2], F32, tag="mv")
    nc.vector.bn_aggr(out=mv, in_=stats)
    # mv[:,1] = 1/sqrt(var+eps)
    nc.scalar.activation(out=mv[:, 1:2], in_=mv[:, 1:2],
                         func=mybir.ActivationFunctionType.Sqrt,
                         bias=pools["eps"][:, 0:1], scale=1.0)
    nc.vector.reciprocal(out=mv[:, 1:2], in_=mv[:, 1:2])
    xn = xp.tile([128, H], BF16, tag="xn")
    nc.vector.tensor_scalar(out=xn, in0=x, scalar1=mv[:, 0:1],
                            scalar2=mv[:, 1:2],
                            op0=mybir.AluOpType.subtract,
                            op1=mybir.AluOpType.mult)
    # transpose 8x [128,128] -> xnt slices, 4 per PSUM bank
    ident = pools["ident"]
    for half in range(2):
        tp = ps.tile([128, 512], BF16, tag="mm",
                     padded_shape=[128, 1024])
        for u in range(4):
            kc = half * 4 + u
            nc.tensor.transpose(tp[:, u * 128:(u + 1) * 128],
                                xn[:, kc * 128:(kc + 1) * 128], ident)
        dst = xnt[:, half * 4:(half + 1) * 4, jslot * 128:(jslot + 1) * 128]
        src = tp.rearrange("p (u t) -> p u t", u=4)
        nc.vector.tensor_copy(out=dst, in_=src)


def _attn_chunk(nc, pools, h, c5, qb, n_sc, KT, QT, V, out_ps):
    """Attention for head h, query block qb, over the 4 s-chunks of
    512-token kv block c5. Accumulates into out_ps [65, 512]."""
    ps, pt_pool = pools["mm"], pools["pt"]
    po = (h % 2) * 64
    mh = h // 2
    for scl in range(0, 4, 2):
        # two 128-token s-chunks share one PSUM tile so exp() runs as a
        # single [128, 1024] ACTIVATE (amortizes the ~350-cycle overhead)
        sc0 = c5 * 4 + scl
        sc_ps = ps.tile([128, 2, 512], F32, tag="mm")
        for i in range(2):
            sc = sc0 + i
            nc.tensor.matmul(
                sc_ps[:, i, :],
                KT[po:po + 64, mh, sc * 128:(sc + 1) * 128],
                QT[po:po + 64, mh, qb * 512:(qb + 1) * 512],
                start=True, stop=True)
        pt = pt_pool.tile([128, 2, 512], BF16, tag="pt")
        nc.scalar.activation(out=pt, in_=sc_ps,
                             func=mybir.ActivationFunctionType.Exp,
                             scale=SCALE)
        for i in range(2):
            sc = sc0 + i
            nc.tensor.matmul(
                out_ps,
                V[:, sc, h, :],
                pt[:, i, :],
                start=(sc == 0), stop=(sc == n_sc - 1))


def build(nq=NQ, s=S):
    n_qb = nq // 512          # query blocks
    n_c5 = s // 512           # kv 512-token chunks
    n_sc = s // 128           # kv 128-token chunks
    shard_rows = 128          # per-RS shard rows per core

    nc = bacc.Bacc("TRN2", target_bir_lowering=False, debug=False,
                   num_devices=8)
    q_d = nc.declare_dram_parameter("q", [nq, H], F32, isOutput=False)
    kv_d = nc.declare_dram_parameter("kv", [s, H], F32, isOutput=False)
    mask_d = nc.declare_dram_parameter("mask", [s], U8, isOutput=False)
    wq_d = nc.declare_dram_parameter("wq", [H, DQ], BF16, isOutput=False)
    wk_d = nc.declare_dram_parameter("wk", [H, DQ], BF16, isOutput=False)
    wv_d = nc.declare_dram_parameter("wv", [H, DQ], BF16, isOutput=False)
    wo_d = nc.declare_dram_parameter("wout", [DQ, H], BF16, isOutput=False)
    bq_d = nc.declare_dram_parameter("bq", [DQ], F32, isOutput=False)
    bk_d = nc.declare_dram_parameter("bk", [DQ], F32, isOutput=False)
    bv_d = nc.declare_dram_parameter("bv", [DQ], F32, isOutput=False)
    bo_d = nc.declare_dram_parameter("bout", [H], F32, isOutput=False)
    # per-qb shards stacked: out[qb*128:(qb+1)*128] = reduced rows
    # [qb*512 + rank*128, +128) of this batch's output
    out_d = nc.declare_dram_parameter("out", [n_qb * shard_rows, H], F32,
                                      isOutput=True)

    part_d = nc.dram_tensor("partial", [nq, H], F32)
    rs_d = nc.dram_tensor("rs_out", [n_qb * shard_rows, H], F32)

    with tile.TileContext(nc) as tc:
        import contextlib
        with contextlib.ExitStack() as ctx:
            singles = ctx.enter_context(tc.tile_pool(name="singles", bufs=1))
            xp = ctx.enter_context(tc.tile_pool(name="x", bufs=2))
            st = ctx.enter_context(tc.tile_pool(name="st", bufs=4))
            ps = ctx.enter_context(
                tc.tile_pool(name="mm", bufs=2, space="PSUM"))
            pv = ctx.enter_context(
                tc.tile_pool(name="pv", bufs=4, space="PSUM"))
            xnt_p = ctx.enter_context(tc.tile_pool(name="xnt", bufs=2))
            pt_p = ctx.enter_context(tc.tile_pool(name="pt", bufs=4))
            misc = ctx.enter_context(tc.tile_pool(name="misc", bufs=2))
            outp = ctx.enter_context(tc.tile_pool(name="outp", bufs=2))

            # ---- constants / weights ----
            ident = singles.tile([128, 128], BF16)
            make_identity(nc, ident)
            eps_t = singles.tile([128, 1], F32)
            nc.vector.memset(eps_t, EPS)
            wq_sb = singles.tile([128, KC, DQ], BF16, tag="wgt")
            nc.sync.dma_start(
                out=wq_sb, in_=wq_d.ap().rearrange("(kc p) n -> p kc n", p=128))
            wk_sb = singles.tile([128, KC, DQ], BF16)
            nc.sync.dma_start(
                out=wk_sb, in_=wk_d.ap().rearrange("(kc p) n -> p kc n", p=128))
            wv_sb = singles.tile([128, KC, DQ], BF16)
            nc.sync.dma_start(
                out=wv_sb, in_=wv_d.ap().rearrange("(kc p) n -> p kc n", p=128))
            bq_sb = singles.tile([128, MQ], F32)
            nc.sync.dma_start(
                out=bq_sb, in_=bq_d.ap().rearrange("(m p) -> p m", p=128))
            bk_sb = singles.tile([128, MQ], F32)
            nc.sync.dma_start(
                out=bk_sb, in_=bk_d.ap().rearrange("(m p) -> p m", p=128))
            bv_row = singles.tile([1, DQ], F32)
            nc.sync.dma_start(out=bv_row, in_=bv_d.ap()[None, :])
            bv_sb = singles.tile([128, DQ], F32)
            nc.gpsimd.partition_broadcast(out_ap=bv_sb, in_ap=bv_row)
            bo_row = singles.tile([1, H], F32)
            nc.sync.dma_start(out=bo_row, in_=bo_d.ap()[None, :])
            bo_sb = singles.tile([128, H], F32)
            nc.gpsimd.partition_broadcast(out_ap=bo_sb, in_ap=bo_row)

            # mask: [s] u8 -> f32 [128, n_sc] (partition = s % 128)
            mask_n8 = singles.tile([n_sc, 128], U8)
            nc.sync.dma_start(
                out=mask_n8,
                in_=mask_d.ap().rearrange("(r c) -> r c", c=128))
            mask_nf = singles.tile([n_sc, 128], BF16)
            nc.vector.tensor_copy(out=mask_nf, in_=mask_n8)
            mask_f = singles.tile([128, n_sc], F32)
            mps = ps.tile([128, n_sc], BF16, tag="mm",
                          padded_shape=[128, 1024])
            nc.tensor.transpose(mps, mask_nf, ident[0:n_sc, 0:n_sc])
            nc.vector.tensor_copy(out=mask_f, in_=mps)

            pools = {"x": xp, "st": st, "mm": ps, "pt": pt_p,
                     "eps": eps_t, "ident": ident}

            # ---- persistent activations ----
            QT = singles.tile([128, MQ, nq], BF16)
            KT = singles.tile([128, MQ, s], BF16)
            V = singles.tile([128, n_sc, NH_L, HD + 1], BF16)
            nc.vector.memset(V[:, :, :, HD:HD + 1], 1.0)
            aoT = singles.tile([128, MQ, nq], BF16)

            # ---- phase A: queries -> QT ----
            for c5 in range(n_qb):
                xnt = xnt_p.tile([128, KC, 512], BF16, tag="xnt")
                for j in range(4):
                    rows = c5 * 512 + j * 128
                    _ln_chunk(nc, pools, q_d.ap()[rows:rows + 128, :], xnt, j)
                for m in range(MQ):
                    mmp = ps.tile([128, 512], F32, tag="mm")
                    for kc in range(KC):
                        nc.tensor.matmul(mmp, wq_sb[:, kc, m * 128:(m + 1) * 128],
                                         xnt[:, kc, :],
                                         start=(kc == 0), stop=(kc == KC - 1))
                    nc.vector.tensor_scalar_add(
                        out=QT[:, m, c5 * 512:(c5 + 1) * 512], in0=mmp,
                        scalar1=bq_sb[:, m:m + 1])

            # ---- phase B: kv chunks -> KT/V, attention qb=0 all heads ----
            out_ps = {h: pv.tile([HD + 1, 512], F32, tag="pv",
                                 name=f"opsA{h}") for h in range(NH_L)}
            for c5 in range(n_c5):
                xnt = xnt_p.tile([128, KC, 512], BF16, tag="xnt")
                for j in range(4):
                    rows = c5 * 512 + j * 128
                    _ln_chunk(nc, pools, kv_d.ap()[rows:rows + 128, :], xnt, j)
                for m in range(MQ):
                    mmp = ps.tile([128, 512], F32, tag="mm")
                    for kc in range(KC):
                        nc.tensor.matmul(mmp, wk_sb[:, kc, m * 128:(m + 1) * 128],
                                         xnt[:, kc, :],
                                         start=(kc == 0), stop=(kc == KC - 1))
                    nc.vector.tensor_scalar_add(
                        out=KT[:, m, c5 * 512:(c5 + 1) * 512], in0=mmp,
                        scalar1=bk_sb[:, m:m + 1])
                for j in range(4):
                    sc = c5 * 4 + j
                    mmp = ps.tile([128, DQ], F32, tag="mm")
                    for kc in range(KC):
                        nc.tensor.matmul(mmp, xnt[:, kc, j * 128:(j + 1) * 128],
                                         wv_sb[:, kc, :],
                                         start=(kc == 0), stop=(kc == KC - 1))
                    nc.vector.tensor_tensor(
                        out=V[:, sc, :, 0:HD],
                        in0=mmp.rearrange("p (h d) -> p h d", h=NH_L),
                        in1=bv_sb.rearrange("p (h d) -> p h d", h=NH_L),
                        op=mybir.AluOpType.add)
                    nc.vector.tensor_scalar_mul(
                        out=V[:, sc, :, :], in0=V[:, sc, :, :],
                        scalar1=mask_f[:, sc:sc + 1])
                for h in range(NH_L):
                    _attn_chunk(nc, pools, h, c5, 0, n_sc, KT, QT, V,
                                out_ps[h])

            def normalize(h, qb, ops):
                po = (h % 2) * 64
                mh = h // 2
                rA = misc.tile([64, 512], F32, tag="rA")
                nc.vector.reciprocal(out=rA[0:1, :], in_=ops[HD:HD + 1, :])
                rB = misc.tile([64, 512], F32, tag="rB")
                nc.gpsimd.partition_broadcast(out_ap=rB, in_ap=rA[0:1, :])
                nc.vector.tensor_tensor(
                    out=aoT[po:po + 64, mh, qb * 512:(qb + 1) * 512],
                    in0=ops[0:HD, :], in1=rB, op=mybir.AluOpType.mult)

            # wo shares the "wgt" slot with wq (wq is dead after phase A)
            wo_sb = singles.tile([128, MQ, H], BF16, tag="wgt")
            nc.sync.dma_start(
                out=wo_sb, in_=wo_d.ap().rearrange("(kc p) n -> p kc n", p=128))

            def outproj_and_rs(qb):
                for mql in range(4):
                    mq = qb * 4 + mql
                    po_t = outp.tile([128, H], F32, tag="po")
                    for nb in range(H // 512):
                        mmp = ps.tile([128, 512], F32, tag="mm")
                        for kc in range(MQ):
                            nc.tensor.matmul(
                                mmp, aoT[:, kc, mq * 128:(mq + 1) * 128],
                                wo_sb[:, kc, nb * 512:(nb + 1) * 512],
                                start=(kc == 0), stop=(kc == MQ - 1))
                        nc.vector.tensor_copy(
                            out=po_t[:, nb * 512:(nb + 1) * 512], in_=mmp)
                    nc.sync.dma_start(
                        out=part_d.ap()[mq * 128:(mq + 1) * 128, :], in_=po_t)
                nc.gpsimd.collective_compute(
                    "ReduceScatter",
                    mybir.AluOpType.add,
                    replica_groups=[[0, 1, 2, 3], [4, 5, 6, 7]],
                    ins=[part_d.ap()[qb * 512:(qb + 1) * 512, :]],
                    outs=[rs_d.ap()[qb * 128:(qb + 1) * 128, :]],
                )

            for h in range(NH_L):
                normalize(h, 0, out_ps[h])
            outproj_and_rs(0)

            # ---- phase C: attention qb=1 (overlaps RS of qb=0) ----
            if n_qb > 1:
                out_ps2 = {h: pv.tile([HD + 1, 512], F32, tag="pv",
                                      name=f"opsB{h}") for h in range(NH_L)}
                for c5 in range(n_c5):
                    for h in range(NH_L):
                        _attn_chunk(nc, pools, h, c5, 1, n_sc, KT, QT, V,
                                    out_ps2[h])
                for h in range(NH_L):
                    normalize(h, 1, out_ps2[h])
                outproj_and_rs(1)

            # ---- add out-proj bias, write shards ----
            for i in range(n_qb):
                t = outp.tile([128, H], F32, tag="fin")
                nc.sync.dma_start(
                    out=t, in_=rs_d.ap()[i * 128:(i + 1) * 128, :])
                nc.vector.tensor_tensor(out=t, in0=t, in1=bo_sb,
                                        op=mybir.AluOpType.add)
                nc.sync.dma_start(
                    out=out_d.ap()[i * 128:(i + 1) * 128, :], in_=t)

    nc.compile()
    return nc


_NC_CACHE = {}


def _get_nc(nq=NQ, s=S):
    key = (nq, s)
    if key not in _NC_CACHE:
        _NC_CACHE[key] = build(nq, s)
    return _NC_CACHE[key]


def make_in_maps(queries, keys_values, attention_mask,
                 W_qkv, b_qkv, W_out, b_out, gamma, beta):
    # exact host-side fold of LN affine params into the qkv projection
    Wf = (W_qkv * gamma[:, None]).astype(np.float32)
    bf = (b_qkv + beta @ W_qkv).astype(np.float32)
    bf16 = ml_dtypes.bfloat16
    in_maps = []
    for c in range(8):
        b = c // GROUPS
        g = c % GROUPS
        sl_q = slice(g * DQ, (g + 1) * DQ)
        sl_k = slice(H + g * DQ, H + (g + 1) * DQ)
        sl_v = slice(2 * H + g * DQ, 2 * H + (g + 1) * DQ)
        in_maps.append({
            "q": np.ascontiguousarray(queries[b]),
            "kv": np.ascontiguousarray(keys_values[b]),
            "mask": np.ascontiguousarray(attention_mask[b]).view(np.uint8),
            "wq": np.ascontiguousarray(Wf[:, sl_q]).astype(bf16),
            "wk": np.ascontiguousarray(Wf[:, sl_k]).astype(bf16),
            "wv": np.ascontiguousarray(Wf[:, sl_v]).astype(bf16),
            "wout": np.ascontiguousarray(
                W_out[g * DQ:(g + 1) * DQ, :]).astype(bf16),
            "bq": np.ascontiguousarray(bf[sl_q]),
            "bk": np.ascontiguousarray(bf[sl_k]),
            "bv": np.ascontiguousarray(bf[sl_v]),
            "bout": np.ascontiguousarray(b_out),
        })
    return in_maps


def kernel(queries, keys_values, attention_mask, W_qkv, b_qkv, W_out, b_out,
           gamma, beta, _trace=False, _nq=NQ, _s=S):
    nc = _get_nc(_nq, _s)
    in_maps = make_in_maps(queries, keys_values, attention_mask,
                           W_qkv, b_qkv, W_out, b_out, gamma, beta)
    res = run_bass_kernel_spmd(nc, in_maps, list(range(8)), trace=_trace)
    n_qb = _nq // 512
    out = np.empty((B, _nq, H), np.float32)
    for c in range(8):
        b = c // GROUPS
        r = c % GROUPS
        shard = res.results[c]["out"]
        for qb in range(n_qb):
            rows = qb * 512 + r * 128
            out[b, rows:rows + 128, :] = shard[qb * 128:(qb + 1) * 128]
    if _trace:
        return out, res
    return out


# revision 21
# speedup vs baseline: 1.1340x; 1.1340x over previous
"""Cross-attention block (LN -> shared qkv proj -> masked softmax attention
-> out proj) on 8 trn2 NeuronCores.

Sharding: 2-way data parallel over batch x 4-way tensor parallel over heads
(16 heads -> 4 per core). LayerNorm params are folded into the qkv weights
host-side (exact: out = LN_affine(x) @ W + b == LN_plain(x) @ (gamma*W) +
(beta @ W + b)). Each core computes a partial out-projection; ReduceScatters
over each 4-core group sum the partials and leave each core with a shard of
its batch's output, which the host reassembles.

Device layout notes:
 - activations live feature-on-partition ("transposed") for all matmuls;
   LayerNorm runs in natural layout and the result is transposed on the PE.
 - scores are computed transposed [s, q] so the softmax reduction over s can
   ride the P@V matmul: V gets an extra all-ones column producing the
   denominator, and masked keys are handled by zeroing their V rows (and the
   ones column), which is exactly softmax with -inf masked scores. exp() is
   applied without max-subtraction (scores for these inputs are O(5), far
   from overflow; softmax is shift-invariant so the result is identical).
 - matmul operands are bf16 (fp32 PSUM accumulate). fp32/f32r matmuls do not
   register as PE activity for the HAM clock governor, pinning the PE at
   1.2 GHz; bf16 runs warm at 2.4 GHz.
 - attention runs one 512-query block at a time across all heads, so the
   out-projection + ReduceScatter of block 0 overlap block 1's attention.
"""

import numpy as np
import ml_dtypes

import concourse.bass as bass
import concourse.mybir as mybir
import concourse.tile as tile
from concourse import bacc
from concourse.bass_utils import run_bass_kernel_spmd
from concourse.masks import make_identity

B, NQ, S, H, NH = 2, 1024, 4096, 1024, 16
HD = H // NH          # 64
GROUPS = 4            # head-parallel ways per batch
NH_L = NH // GROUPS   # heads per core
DQ = NH_L * HD        # per-core projected dim (256)
EPS = 1e-6
SCALE = 1.0 / float(np.sqrt(HD))

F32 = mybir.dt.float32
BF16 = mybir.dt.bfloat16
U8 = mybir.dt.uint8

KC = H // 128         # feature chunks (8)
MQ = DQ // 128        # per-core projected-dim tiles (2)


def _ln_chunk(nc, pools, x_dram_rows, xnt, jslot):
    """LayerNorm 128 tokens (natural layout) then transpose into
    xnt[:, :, jslot*128:(jslot+1)*128] (feature-on-partition, bf16)."""
    xp, st, ps = pools["x"], pools["st"], pools["mm"]
    x = xp.tile([128, H], F32, tag="x")
    nc.sync.dma_start(out=x, in_=x_dram_rows)
    stats = st.tile([128, 2, 6], F32, tag="st")
    nc.vector.bn_stats(out=stats[:, 0, :], in_=x[:, 0:512])
    nc.vector.bn_stats(out=stats[:, 1, :], in_=x[:, 512:1024])
    mv = st.tile([128, 2], F32, tag="mv")
    nc.vector.bn_aggr(out=mv, in_=stats)
    # mv[:,1] = 1/sqrt(var+eps) computed as exp(-0.5*ln(var+eps)): Ln and Exp
    # share one ACT table set with the attention exp, avoiding ~1.5us
    # ACT_TABLE_LOADs on every LN<->softmax switch
    nc.scalar.activation(out=mv[:, 1:2], in_=mv[:, 1:2],
                         func=mybir.ActivationFunctionType.Ln,
                         bias=pools["eps"][:, 0:1], scale=1.0)
    nc.scalar.activation(out=mv[:, 1:2], in_=mv[:, 1:2],
                         func=mybir.ActivationFunctionType.Exp,
                         scale=-0.5)
    xn = xp.tile([128, H], BF16, tag="xn")
    nc.vector.tensor_scalar(out=xn, in0=x, scalar1=mv[:, 0:1],
                            scalar2=mv[:, 1:2],
                            op0=mybir.AluOpType.subtract,
                            op1=mybir.AluOpType.mult)
    # transpose 8x [128,128] -> xnt slices, 4 per PSUM bank
    ident = pools["ident"]
    for half in range(2):
        tp = ps.tile([128, 512], BF16, tag="mm",
                     padded_shape=[128, 1024])
        for u in range(4):
            kc = half * 4 + u
            nc.tensor.transpose(tp[:, u * 128:(u + 1) * 128],
                                xn[:, kc * 128:(kc + 1) * 128], ident)
        dst = xnt[:, half * 4:(half + 1) * 4, jslot * 128:(jslot + 1) * 128]
        src = tp.rearrange("p (u t) -> p u t", u=4)
        nc.vector.tensor_copy(out=dst, in_=src)


class AttnPipe:
    """Software-pipelined attention: keeps >=2 score/exp pairs in flight so
    the PE never stalls waiting on the ACT exp (exp of an sc-pair takes
    ~1.15us while the matching 4 matmuls take ~0.9us). Depth-2 lets the PE
    run two score pairs ahead of the P@V consumers."""

    DEPTH = 2

    def __init__(self, nc, pools, qb, n_sc, KT, QT, V, out_ps):
        self.nc, self.pools = nc, pools
        self.qb, self.n_sc = qb, n_sc
        self.KT, self.QT, self.V, self.out_ps = KT, QT, V, out_ps
        self.pend = []

    def emit(self, h, sc0):
        nc, (ps, pt_pool) = self.nc, (self.pools["mm"], self.pools["pt"])
        po = (h % 2) * 64
        mh = h // 2
        sc_ps = ps.tile([128, 2, 512], F32, tag="mm", name="sc_ps")
        for i in range(2):
            sc = sc0 + i
            nc.tensor.matmul(
                sc_ps[:, i, :],
                self.KT[po:po + 64, mh, sc * 128:(sc + 1) * 128],
                self.QT[po:po + 64, mh,
                        self.qb * 512:(self.qb + 1) * 512],
                start=True, stop=True)
        pt = pt_pool.tile([128, 2, 512], BF16, tag="pt", name="pt")
        nc.scalar.activation(out=pt, in_=sc_ps,
                             func=mybir.ActivationFunctionType.Exp,
                             scale=SCALE)
        self.pend.append((pt, h, sc0))
        if len(self.pend) > self.DEPTH:
            self._drain_one()

    def _drain_one(self):
        nc = self.nc
        pt, h, sc0 = self.pend.pop(0)
        for i in range(2):
            sc = sc0 + i
            nc.tensor.matmul(
                self.out_ps[h],
                self.V[:, sc, h, :],
                pt[:, i, :],
                start=(sc == 0), stop=(sc == self.n_sc - 1))

    def flush(self):
        while self.pend:
            self._drain_one()


def build(nq=NQ, s=S):
    n_qb = nq // 512          # query blocks
    n_c5 = s // 512           # kv 512-token chunks
    n_sc = s // 128           # kv 128-token chunks
    shard_rows = 128          # per-RS shard rows per core

    nc = bacc.Bacc("TRN2", target_bir_lowering=False, debug=False,
                   num_devices=8)
    q_d = nc.declare_dram_parameter("q", [nq, H], F32, isOutput=False)
    kv_d = nc.declare_dram_parameter("kv", [s, H], F32, isOutput=False)
    mask_d = nc.declare_dram_parameter("mask", [s], U8, isOutput=False)
    wq_d = nc.declare_dram_parameter("wq", [H, DQ], BF16, isOutput=False)
    wk_d = nc.declare_dram_parameter("wk", [H, DQ], BF16, isOutput=False)
    wv_d = nc.declare_dram_parameter("wv", [H, DQ], BF16, isOutput=False)
    wo_d = nc.declare_dram_parameter("wout", [DQ, H], BF16, isOutput=False)
    bq_d = nc.declare_dram_parameter("bq", [DQ], F32, isOutput=False)
    bk_d = nc.declare_dram_parameter("bk", [DQ], F32, isOutput=False)
    bv_d = nc.declare_dram_parameter("bv", [DQ], F32, isOutput=False)
    bo_d = nc.declare_dram_parameter("bout", [H], F32, isOutput=False)
    # per-qb shards stacked: out[qb*128:(qb+1)*128] = reduced rows
    # [qb*512 + rank*128, +128) of this batch's output
    out_d = nc.declare_dram_parameter("out", [n_qb * shard_rows, H], F32,
                                      isOutput=True)

    part_d = nc.dram_tensor("partial", [nq, H], F32)
    rs_d = nc.dram_tensor("rs_out", [n_qb * shard_rows, H], F32)

    with tile.TileContext(nc) as tc:
        import contextlib
        with contextlib.ExitStack() as ctx:
            singles = ctx.enter_context(tc.tile_pool(name="singles", bufs=1))
            xp = ctx.enter_context(tc.tile_pool(name="x", bufs=2))
            st = ctx.enter_context(tc.tile_pool(name="st", bufs=4))
            ps = ctx.enter_context(
                tc.tile_pool(name="mm", bufs=3, space="PSUM"))
            pv = ctx.enter_context(
                tc.tile_pool(name="pv", bufs=2, space="PSUM"))
            xnt_p = ctx.enter_context(tc.tile_pool(name="xnt", bufs=2))
            pt_p = ctx.enter_context(tc.tile_pool(name="pt", bufs=4))
            misc = ctx.enter_context(tc.tile_pool(name="misc", bufs=2))
            outp = ctx.enter_context(tc.tile_pool(name="outp", bufs=2))

            # ---- constants / weights ----
            ident = singles.tile([128, 128], BF16)
            make_identity(nc, ident)
            eps_t = singles.tile([128, 1], F32)
            nc.vector.memset(eps_t, EPS)
            wq_sb = singles.tile([128, KC, DQ], BF16, tag="wgt")
            nc.sync.dma_start(
                out=wq_sb, in_=wq_d.ap().rearrange("(kc p) n -> p kc n", p=128))
            wk_sb = singles.tile([128, KC, DQ], BF16)
            nc.sync.dma_start(
                out=wk_sb, in_=wk_d.ap().rearrange("(kc p) n -> p kc n", p=128))
            wv_sb = singles.tile([128, KC, DQ], BF16)
            nc.sync.dma_start(
                out=wv_sb, in_=wv_d.ap().rearrange("(kc p) n -> p kc n", p=128))
            bq_sb = singles.tile([128, MQ], F32)
            nc.sync.dma_start(
                out=bq_sb, in_=bq_d.ap().rearrange("(m p) -> p m", p=128))
            bk_sb = singles.tile([128, MQ], F32)
            nc.sync.dma_start(
                out=bk_sb, in_=bk_d.ap().rearrange("(m p) -> p m", p=128))
            bv_row = singles.tile([1, DQ], F32)
            nc.sync.dma_start(out=bv_row, in_=bv_d.ap()[None, :])
            bv_sb = singles.tile([128, DQ], F32)
            nc.gpsimd.partition_broadcast(out_ap=bv_sb, in_ap=bv_row)
            bo_row = singles.tile([1, H], F32)
            nc.sync.dma_start(out=bo_row, in_=bo_d.ap()[None, :])
            bo_sb = singles.tile([128, H], F32)
            nc.gpsimd.partition_broadcast(out_ap=bo_sb, in_ap=bo_row)

            # mask: [s] u8 -> f32 [128, n_sc] (partition = s % 128)
            mask_n8 = singles.tile([n_sc, 128], U8)
            nc.sync.dma_start(
                out=mask_n8,
                in_=mask_d.ap().rearrange("(r c) -> r c", c=128))
            mask_nf = singles.tile([n_sc, 128], BF16)
            nc.vector.tensor_copy(out=mask_nf, in_=mask_n8)
            mask_f = singles.tile([128, n_sc], F32)
            mps = ps.tile([128, n_sc], BF16, tag="mm",
                          padded_shape=[128, 1024])
            nc.tensor.transpose(mps, mask_nf, ident[0:n_sc, 0:n_sc])
            nc.vector.tensor_copy(out=mask_f, in_=mps)

            pools = {"x": xp, "st": st, "mm": ps, "pt": pt_p,
                     "eps": eps_t, "ident": ident}

            # ---- persistent activations ----
            QT = singles.tile([128, MQ, nq], BF16)
            KT = singles.tile([128, MQ, s], BF16)
            V = singles.tile([128, n_sc, NH_L, HD + 1], BF16)
            nc.vector.memset(V[:, :, :, HD:HD + 1], 1.0)
            aoT = singles.tile([128, MQ, nq], BF16)

            # ---- phase A: queries -> QT ----
            for c5 in range(n_qb):
                xnt = xnt_p.tile([128, KC, 512], BF16, tag="xnt")
                for j in range(4):
                    rows = c5 * 512 + j * 128
                    _ln_chunk(nc, pools, q_d.ap()[rows:rows + 128, :], xnt, j)
                for m in range(MQ):
                    mmp = ps.tile([128, 512], F32, tag="mm")
                    for kc in range(KC):
                        nc.tensor.matmul(mmp, wq_sb[:, kc, m * 128:(m + 1) * 128],
                                         xnt[:, kc, :],
                                         start=(kc == 0), stop=(kc == KC - 1))
                    nc.vector.tensor_scalar_add(
                        out=QT[:, m, c5 * 512:(c5 + 1) * 512], in0=mmp,
                        scalar1=bq_sb[:, m:m + 1])

            def kv_chunk(c5):
                xnt = xnt_p.tile([128, KC, 512], BF16, tag="xnt",
                                 name="xnt")
                for j in range(4):
                    rows = c5 * 512 + j * 128
                    _ln_chunk(nc, pools, kv_d.ap()[rows:rows + 128, :], xnt, j)
                for m in range(MQ):
                    mmp = ps.tile([128, 512], F32, tag="mm", name="kps")
                    for kc in range(KC):
                        nc.tensor.matmul(mmp, wk_sb[:, kc, m * 128:(m + 1) * 128],
                                         xnt[:, kc, :],
                                         start=(kc == 0), stop=(kc == KC - 1))
                    nc.vector.tensor_scalar_add(
                        out=KT[:, m, c5 * 512:(c5 + 1) * 512], in0=mmp,
                        scalar1=bk_sb[:, m:m + 1])
                for j in range(4):
                    sc = c5 * 4 + j
                    mmp = ps.tile([128, DQ], F32, tag="mm", name="vps")
                    for kc in range(KC):
                        nc.tensor.matmul(mmp, xnt[:, kc, j * 128:(j + 1) * 128],
                                         wv_sb[:, kc, :],
                                         start=(kc == 0), stop=(kc == KC - 1))
                    nc.vector.tensor_tensor(
                        out=V[:, sc, :, 0:HD],
                        in0=mmp.rearrange("p (h d) -> p h d", h=NH_L),
                        in1=bv_sb.rearrange("p (h d) -> p h d", h=NH_L),
                        op=mybir.AluOpType.add)
                    nc.vector.tensor_scalar_mul(
                        out=V[:, sc, :, :], in0=V[:, sc, :, :],
                        scalar1=mask_f[:, sc:sc + 1])

            def attn_phase(hpair, qb, weave_kv):
                """Attention for heads {2*hpair, 2*hpair+1}, query block qb,
                over all kv chunks; produces KT/V on the way if weave_kv."""
                hs = (2 * hpair, 2 * hpair + 1)
                ops = {h: pv.tile([HD + 1, 512], F32, tag="pv",
                                  name=f"ops{h}_{qb}") for h in hs}
                pipe = AttnPipe(nc, pools, qb, n_sc, KT, QT, V, ops)
                for c5 in range(n_c5):
                    if weave_kv:
                        kv_chunk(c5)
                    for h in hs:
                        for pairidx in range(2):
                            pipe.emit(h, c5 * 4 + 2 * pairidx)
                pipe.flush()
                for h in hs:
                    normalize(h, qb, ops[h])

            def normalize(h, qb, ops):
                po = (h % 2) * 64
                mh = h // 2
                rA = misc.tile([64, 512], F32, tag="rA")
                nc.vector.reciprocal(out=rA[0:1, :], in_=ops[HD:HD + 1, :])
                rB = misc.tile([64, 512], F32, tag="rB")
                nc.gpsimd.partition_broadcast(out_ap=rB, in_ap=rA[0:1, :])
                nc.vector.tensor_tensor(
                    out=aoT[po:po + 64, mh, qb * 512:(qb + 1) * 512],
                    in0=ops[0:HD, :], in1=rB, op=mybir.AluOpType.mult)

            # wo shares the "wgt" slot with wq (wq is dead after phase A)
            wo_sb = singles.tile([128, MQ, H], BF16, tag="wgt")
            nc.sync.dma_start(
                out=wo_sb, in_=wo_d.ap().rearrange("(kc p) n -> p kc n", p=128))

            def outproj_and_rs(qb):
                for mql in range(4):
                    mq = qb * 4 + mql
                    po_t = outp.tile([128, H], F32, tag="po")
                    for nb in range(H // 512):
                        mmp = ps.tile([128, 512], F32, tag="mm")
                        for kc in range(MQ):
                            nc.tensor.matmul(
                                mmp, aoT[:, kc, mq * 128:(mq + 1) * 128],
                                wo_sb[:, kc, nb * 512:(nb + 1) * 512],
                                start=(kc == 0), stop=(kc == MQ - 1))
                        nc.vector.tensor_copy(
                            out=po_t[:, nb * 512:(nb + 1) * 512], in_=mmp)
                    nc.sync.dma_start(
                        out=part_d.ap()[mq * 128:(mq + 1) * 128, :], in_=po_t)
                nc.gpsimd.collective_compute(
                    "ReduceScatter",
                    mybir.AluOpType.add,
                    replica_groups=[[0, 1, 2, 3], [4, 5, 6, 7]],
                    ins=[part_d.ap()[qb * 512:(qb + 1) * 512, :]],
                    outs=[rs_d.ap()[qb * 128:(qb + 1) * 128, :]],
                )

            # phase B: kv proj woven into heads 0-1 of qb=0; then heads 2-3;
            # then outproj+RS of qb=0 overlaps qb=1's attention phases
            attn_phase(0, 0, weave_kv=True)
            attn_phase(1, 0, weave_kv=False)
            outproj_and_rs(0)
            if n_qb > 1:
                attn_phase(0, 1, weave_kv=False)
                attn_phase(1, 1, weave_kv=False)
                outproj_and_rs(1)

            # ---- add out-proj bias, write shards ----
            for i in range(n_qb):
                t = outp.tile([128, H], F32, tag="fin")
                nc.sync.dma_start(
                    out=t, in_=rs_d.ap()[i * 128:(i + 1) * 128, :])
                nc.vector.tensor_tensor(out=t, in0=t, in1=bo_sb,
                                        op=mybir.AluOpType.add)
                nc.sync.dma_start(
                    out=out_d.ap()[i * 128:(i + 1) * 128, :], in_=t)

    nc.compile()
    return nc


_NC_CACHE = {}


def _get_nc(nq=NQ, s=S):
    key = (nq, s)
    if key not in _NC_CACHE:
        _NC_CACHE[key] = build(nq, s)
    return _NC_CACHE[key]


def make_in_maps(queries, keys_values, attention_mask,
                 W_qkv, b_qkv, W_out, b_out, gamma, beta):
    # exact host-side fold of LN affine params into the qkv projection
    Wf = (W_qkv * gamma[:, None]).astype(np.float32)
    bf = (b_qkv + beta @ W_qkv).astype(np.float32)
    bf16 = ml_dtypes.bfloat16
    in_maps = []
    for c in range(8):
        b = c // GROUPS
        g = c % GROUPS
        sl_q = slice(g * DQ, (g + 1) * DQ)
        sl_k = slice(H + g * DQ, H + (g + 1) * DQ)
        sl_v = slice(2 * H + g * DQ, 2 * H + (g + 1) * DQ)
        in_maps.append({
            "q": np.ascontiguousarray(queries[b]),
            "kv": np.ascontiguousarray(keys_values[b]),
            "mask": np.ascontiguousarray(attention_mask[b]).view(np.uint8),
            "wq": np.ascontiguousarray(Wf[:, sl_q]).astype(bf16),
            "wk": np.ascontiguousarray(Wf[:, sl_k]).astype(bf16),
            "wv": np.ascontiguousarray(Wf[:, sl_v]).astype(bf16),
            "wout": np.ascontiguousarray(
                W_out[g * DQ:(g + 1) * DQ, :]).astype(bf16),
            "bq": np.ascontiguousarray(bf[sl_q]),
            "bk": np.ascontiguousarray(bf[sl_k]),
            "bv": np.ascontiguousarray(bf[sl_v]),
            "bout": np.ascontiguousarray(b_out),
        })
    return in_maps


def kernel(queries, keys_values, attention_mask, W_qkv, b_qkv, W_out, b_out,
           gamma, beta, _trace=False, _nq=NQ, _s=S):
    nc = _get_nc(_nq, _s)
    in_maps = make_in_maps(queries, keys_values, attention_mask,
                           W_qkv, b_qkv, W_out, b_out, gamma, beta)
    res = run_bass_kernel_spmd(nc, in_maps, list(range(8)), trace=_trace)
    n_qb = _nq // 512
    out = np.empty((B, _nq, H), np.float32)
    for c in range(8):
        b = c // GROUPS
        r = c % GROUPS
        shard = res.results[c]["out"]
        for qb in range(n_qb):
            rows = qb * 512 + r * 128
            out[b, rows:rows + 128, :] = shard[qb * 128:(qb + 1) * 128]
    if _trace:
        return out, res
    return out
